# revision 34
# baseline (speedup 1.0000x reference)
"""Trainium2 Bass kernel for nn_Model_1580547969651.

Math (from the reference):
    s    = x @ sum(y, axis=0)          # (B,) row-sums of x @ y^T
    h    = hardswish(s)                # s * clip(s+3, 0, 6) / 6
    out  = clip(h + noise, -0.5, 0.5)  # (B, 1)

Strategy: COLUMN-shard x and y across the 8 cores (512 features each).
Each core's column-sum of its y shard is locally complete, so there is
no mid-kernel collective. y streams in (s p c)-packed so every DMA
descriptor covers a contiguous 16KB per partition (split across both
HWDGE rings); the VectorEngine folds each 2MB super-tile into a
(128, 512) accumulator as it lands. One ones(128,128) matmul then does
the partition-sum AND the 128-way broadcast in one shot. Phase B
computes partial dots s_i = x[:, F_i] @ ysum_i for ALL 8192 rows with
fused scalar_tensor_tensor ops while x streams (host pre-permutes x
rows so this layout still produces batch-ordered partials). The
partials are transposed on the VectorEngine (32x32 blocks) so the
collective bounce DMA is contiguous; one 32KB AllReduce (Mesh - faster
and lower-variance than ReduceScatter's RDH at this size) sums the
partials, every core runs the cheap elementwise tail on all 8192 rows
in a DMA-friendly (64, 128) layout, and the host keeps each core's
1024-row shard at gather time. A dummy 32B AllReduce issued up front
absorbs the ncfw wake-up / entry rendezvous while the streams run, so
the real AllReduce starts with ~1us instead of ~11.5us trigger delay.
"""

import numpy as np

from concourse import bass, bacc, mybir, tile
from concourse.bass_utils import run_bass_kernel_spmd

B = 8192
F = 4096
NCORES = 8
FL = F // NCORES        # 512 features per core
BL = B // NCORES        # 1024 output rows per core
NST = 8                 # y/x super-tiles (128 part x 8 subtiles x 512)
NSUB = 8                # subtiles per super-tile
NT = NST * NSUB         # 64 (128-row) tiles covering all 8192 rows
FP32 = mybir.dt.float32

_CACHE: dict = {}


def _build():
    nc = bacc.Bacc(
        "TRN2",
        target_bir_lowering=False,
        debug=False,
        num_devices=NCORES,
    )

    x_d = nc.dram_tensor("x", [B, FL], FP32, kind="ExternalInput")
    y_d = nc.dram_tensor("y", [B, FL], FP32, kind="ExternalInput")
    nz_d = nc.dram_tensor("noise", [B, 1], FP32, kind="ExternalInput")
    out_d = nc.dram_tensor("out", [B, 1], FP32, kind="ExternalOutput")

    # (s p c) packing: partition p's slice of super-tile s is 8 consecutive
    # DRAM rows = one contiguous 16KB chunk per descriptor.
    y_r = y_d[:, :].rearrange("(s p c) f -> s p c f", p=128, c=NSUB)
    x_r = x_d[:, :].rearrange("(s p c) f -> s p c f", p=128, c=NSUB)
    nz_r = nz_d[:, 0].rearrange("(k p) -> k p", p=128)      # (64, 128) contig
    out_r = out_d[:, 0].rearrange("(k p) -> k p", p=128)    # (64, 128) contig

    with tile.TileContext(nc) as tc:
        with (
            tc.tile_pool(name="ypool", bufs=5) as ypool,
            tc.tile_pool(name="xpool", bufs=5) as xpool,
            tc.tile_pool(name="small", bufs=1) as small,
            tc.tile_pool(name="scratch", bufs=2) as scratch,
            tc.tile_pool(name="psum", bufs=1, space="PSUM") as psum,
            tc.tile_pool(name="dram", bufs=1, space="DRAM") as dram,
        ):
            ones128 = small.tile([128, 128], FP32)
            nc.gpsimd.memset(ones128[:], 1.0)

            # tiny dummy collective, issued up front: pays the ncfw wake +
            # entry rendezvous while the streams run, so the real AllReduce
            # at the end starts without the ~11.5us first-op delay
            warm = small.tile([1, 8], FP32)
            nc.gpsimd.memset(warm[:], 0.0)
            warm_in = dram.tile([8], FP32)
            warm_out = dram.tile([8], FP32)
            nc.gpsimd.dma_start(warm_in[:], warm[:])
            nc.gpsimd.collective_compute(
                "AllReduce",
                mybir.AluOpType.add,
                replica_groups=[list(range(NCORES))],
                ins=[warm_in.opt()],
                outs=[warm_out.opt()],
            )

            # noise is only needed at the very end; load it now so the
            # gpsimd queue isn't fetching it behind the AllReduce
            noise_t = small.tile([NT, 128], FP32)
            nc.gpsimd.dma_start(noise_t[:], nz_r)

            # ---- phase A: reduce each y super-tile as it lands, split
            # between DVE (subtiles 0-3, folded into acc) and the idle
            # TensorEngine (subtiles 4-7 fed raw into the accumulating
            # broadcast matmul group) ----
            acc = small.tile([128, FL], FP32)
            bc = psum.tile([128, FL], FP32, tag="bc")
            for s in range(NST):
                ytile = ypool.tile([128, NSUB, FL], FP32, tag="y")
                nc.sync.dma_start(ytile[:, 0:NSUB // 2, :],
                                  y_r[s, :, 0:NSUB // 2, :])
                nc.scalar.dma_start(ytile[:, NSUB // 2:, :],
                                    y_r[s, :, NSUB // 2:, :])
                # DVE: fold subtiles 0-3 into acc
                nc.vector.tensor_add(ytile[:, 0:2, :], ytile[:, 0:2, :],
                                     ytile[:, 2:4, :])
                if s == 0:
                    nc.vector.tensor_tensor(
                        out=acc[:], in0=ytile[:, 0, :], in1=ytile[:, 1, :],
                        op=mybir.AluOpType.add)
                else:
                    nc.vector.tensor_add(acc[:], acc[:], ytile[:, 0, :])
                    nc.vector.tensor_add(acc[:], acc[:], ytile[:, 1, :])
                # PE: bc[q, f] += sum_p ones[p, q] * ytile[p, c, f]
                for c in range(NSUB // 2, NSUB):
                    nc.tensor.matmul(bc[:], ones128[:], ytile[:, c, :],
                                     start=(s == 0 and c == NSUB // 2),
                                     stop=False)
            # fold the DVE accumulator in last (partition-sum + broadcast
            # land in bc together)
            nc.tensor.matmul(bc[:], ones128[:], acc[:],
                             start=False, stop=True)

            # ---- phase B: partial dots for ALL rows while x streams ----
            s_part = small.tile([128, NT], FP32)
            s_t = small.tile([64, 128], FP32)
            for s in range(NST):
                xtile = xpool.tile([128, NSUB, FL], FP32, tag="x")
                # last super-tile: 6/2 split so only 2 subtiles trail the
                # final arrival
                cut = NSUB // 2 if s < NST - 1 else 6
                nc.sync.dma_start(xtile[:, 0:cut, :], x_r[s, :, 0:cut, :])
                nc.scalar.dma_start(xtile[:, cut:, :], x_r[s, :, cut:, :])
                for t in range(NSUB):
                    m = s * NSUB + t
                    prod = scratch.tile([128, FL], FP32, tag="sc")
                    nc.vector.scalar_tensor_tensor(
                        out=prod[:],
                        in0=xtile[:, t, :],
                        scalar=1.0,
                        in1=bc[:],
                        op0=mybir.AluOpType.mult,
                        op1=mybir.AluOpType.mult,
                        accum_out=s_part[:, m:m + 1],
                    )
                if s == NST // 2 - 1:
                    # columns 0..31 are complete: transpose them now,
                    # overlapped with the rest of the stream
                    for i in range(4):
                        nc.vector.transpose(
                            s_t[0:32, 32 * i:32 * (i + 1)],
                            s_part[32 * i:32 * (i + 1), 0:32],
                        )

            # ---- transpose the remaining s_part columns (32x32 blocks)
            # so the AllReduce bounce DMA is contiguous ----
            for i in range(4):
                nc.vector.transpose(
                    s_t[32:64, 32 * i:32 * (i + 1)],
                    s_part[32 * i:32 * (i + 1), 32:64],
                )

            # ---- AllReduce the 32KB of partials (Mesh; faster + less
            # variance than a ReduceScatter's RDH at this size). Every
            # core computes the full tail; the host slices its shard. ----
            cc_in = dram.tile([B], FP32)
            cc_out = dram.tile([B], FP32)
            nc.gpsimd.dma_start(cc_in[:].rearrange("(m p) -> m p", p=128),
                                s_t[:])
            nc.gpsimd.collective_compute(
                "AllReduce",
                mybir.AluOpType.add,
                replica_groups=[list(range(NCORES))],
                ins=[cc_in.opt()],
                outs=[cc_out.opt()],
            )
            s_mine = small.tile([NT, 128], FP32)
            nc.gpsimd.dma_start(s_mine[:],
                                cc_out[:].rearrange("(k p) -> k p", p=128))

            # ---- tail: hardswish, + noise, hardtanh (in (64,128) layout) ----
            t_ = small.tile([NT, 128], FP32)
            nc.vector.tensor_scalar(
                out=t_[:], in0=s_mine[:], scalar1=3.0, scalar2=0.0,
                op0=mybir.AluOpType.add, op1=mybir.AluOpType.max,
            )
            nc.vector.tensor_scalar(
                out=t_[:], in0=t_[:], scalar1=6.0, scalar2=1.0 / 6.0,
                op0=mybir.AluOpType.min, op1=mybir.AluOpType.mult,
            )
            r = small.tile([NT, 128], FP32)
            nc.vector.tensor_tensor(
                out=r[:], in0=s_mine[:], in1=t_[:], op=mybir.AluOpType.mult,
            )
            nc.vector.tensor_tensor(
                out=r[:], in0=r[:], in1=noise_t[:], op=mybir.AluOpType.add,
            )
            nc.vector.tensor_scalar(
                out=r[:], in0=r[:], scalar1=-0.5, scalar2=0.5,
                op0=mybir.AluOpType.max, op1=mybir.AluOpType.min,
            )
            nc.gpsimd.dma_start(out_r, r[:])

    nc.compile()
    return nc


def _get_nc():
    if "nc" not in _CACHE:
        _CACHE["nc"] = _build()
    return _CACHE["nc"]


# device row (s p c) -> global row 128*(8s+c)+p, so that s_part column
# m = 8s+c, partition p lands on global row 128m+p (what the RS expects)
def _permute_rows(a: np.ndarray) -> np.ndarray:
    # a: (8192, cols); view as (s, c, p, cols), want (s, p, c, cols)
    return np.ascontiguousarray(
        a.reshape(NST, NSUB, 128, a.shape[1]).transpose(0, 2, 1, 3)
        .reshape(B, a.shape[1])
    )


def kernel(x: np.ndarray, y: np.ndarray, noise: np.ndarray, **_run_kwargs) -> np.ndarray:
    x = np.ascontiguousarray(x, dtype=np.float32)
    y = np.ascontiguousarray(y, dtype=np.float32)
    noise = np.ascontiguousarray(noise, dtype=np.float32)

    nc = _get_nc()
    xp = _permute_rows(x)
    in_maps = [
        {
            "x": np.ascontiguousarray(xp[:, i * FL:(i + 1) * FL]),
            "y": np.ascontiguousarray(y[:, i * FL:(i + 1) * FL]),
            "noise": noise,
        }
        for i in range(NCORES)
    ]
    res = run_bass_kernel_spmd(nc, in_maps, list(range(NCORES)), **_run_kwargs)
    out = np.concatenate(
        [res.results[i]["out"][i * BL:(i + 1) * BL] for i in range(NCORES)],
        axis=0,
    )
    if _run_kwargs:
        _CACHE["last_results"] = res
    return out
